# revision 33
# baseline (speedup 1.0000x reference)
"""Trainium2 Bass kernel for nn_Encoder segment-reduce.

Reference computation (per sample b):
    cls = onehot(argmax_k outputs[b])            # [K, HW]
    sizes = cls.sum(HW) + 0.01                   # [K]
    feat_set = feats[b] @ cls.T / sizes          # [F, K]
    out[b] = w_proj @ feat_set + bias            # [E, K]

Kernel strategy (pure data parallel: 1 sample per NeuronCore, 8 cores).

Segment-reduce FIRST (the cheap contraction), projection second:
    feat_setT[k, f] = sum_hw onehot[hw, k] * featsT[hw, f]
with the onehot chunk [128hw, 21] as the PE's stationary operand and featsT
chunks [128hw, 512f] as the moving operand.  The four f-group matmuls of each
hw chunk are packed into the four 32-column groups of the PE array via
tile_position=(0, 32j): the stationary onehot only occupies 21 of 128 array
columns, so the four matmuls execute concurrently (measured 4ns stagger) and
the stream keeps pace with the DMA.  One [128, 512] PSUM tile holds all four
accumulators (f-group j at partitions 32j..32j+21).

feats dtype is fp8 e3m4 (TRN FP8_EXP3): N(0,1) data fits the e3m4 range and
its 4 mantissa bits give rel err ~1.2e-2 end to end (threshold 2e-2), halving
HBM traffic vs bf16: 8.39 MB feats + 1 MB wT (bf16) + 0.34 MB outputs (f32)
~= 9.75 MB/core -- the kernel is DMA-bound at the per-core HBM limit.  The
matmuls run in normal (single-rate) fp8 mode: double-fp8 would upcast
operands to e6m3 and destroy e3m4's 4th mantissa bit (measured 3.0e-2).

All host-side layouts give every DMA >= 2KB contiguous per-partition runs
(wT is pre-permuted to [p, fc, e]; bias rides in the outputs transfer).
DMA order: outputs+bias first (the DVE argmax fills the initial feats
window), feats in 1-2MB blocks, the first quarter of wT (needed by the first
projection round) just before the last feats block, then the rest of wT.
The per-chunk size-count matmul is interleaved with the stream so the PE
never serializes behind the (DVE-paced) argmax; the warm-up burst is sized
to end just as the first feats block lands (a longer burst delays the
stream: the PE executes in order at the cold 1.2 GHz clock).

Tail (all PE stages packed into array tile groups, dummy-matmul fillers in
the dependency gaps so the HAM clock gate keeps the PE at 2.4 GHz):
  - 1/sizes is replicated to all four column groups with one bf16 matmul,
    then fused into the PSUM->SBUF copies (DVE + ACT halves);
  - the 16 [21,128]->[128,21] transposes run 4-at-a-time (one per 32-row
    row group, via a replicated identity) into ONE [128, 16*21] PSUM tile,
    drained by two half-copies (DVE + ACT);
  - the 16 projection matmuls (feat_set chunk stationary, wT moving) run
    4-at-a-time in the four column groups (round r takes f-chunks 4r..4r+3),
    giving four partial [21, 256] sums at partitions 32j; one final matmul
    against the replicated identity combines them, with the bias pre-loaded
    into its PSUM accumulator early via a diagonal-bias matmul.
The output is stored as [K, E]; the host transposes when gathering.

dtype: "fp8" (e3m4 feats, rel err ~1.2e-2) or "bf16" (rel err ~3e-3).
"""

import numpy as np

import concourse.bacc as bacc
import concourse.bass as bass
import concourse.mybir as mybir
import concourse.tile as tile
from concourse.bass import ds, ts
from concourse.bass_utils import run_bass_kernel_spmd
from concourse.masks import make_identity

# Problem shapes (hardcoded per contract)
B = 8
K = 21
H = 64
W = 64
HW = H * W            # 4096
F = 2048
E = 256
P = 128
FC = F // P           # 16 f-chunks of 128
FG = 4                # f-groups of 512 (PE column groups)
FGW = F // FG         # 512
N_T = HW // P         # 32 hw chunks
OUT_AUG = N_T * K + 2  # outputs row + 2 bias values per partition
N_CORES = 8

F32 = mybir.dt.float32
BF16 = mybir.dt.bfloat16
FP8 = mybir.dt.float8e3   # e3m4: 4 mantissa bits

DTYPE = "fp8"         # "fp8" or "bf16"


def build_module(dtype=DTYPE):
    mm_dt = FP8 if dtype == "fp8" else BF16
    nc = bacc.Bacc("TRN2", target_bir_lowering=False, debug=False)

    # outputs host-transposed to [p, t*k] (pixel-major), with the two bias
    # values of partition p (bias[p], bias[128+p]) appended -- one transfer.
    outputs_d = nc.dram_tensor("outputs_in", [P, OUT_AUG], F32, kind="ExternalInput")
    # featsT host-permuted to [p, t, fgrp, fj]: featsT[t*128+p, fgrp*512+fj].
    feats_d = nc.dram_tensor(
        "feats_in", [P, N_T, FG, FGW], mm_dt, kind="ExternalInput"
    )
    # wT host-permuted to [p, fc, e] = w_proj.T[fc*128+p, e] (contiguous 8KB
    # per-partition runs -- the naive [F, E] layout DMAs in 512B pieces).
    wT_d = nc.dram_tensor("wT_in", [P, FC, E], BF16, kind="ExternalInput")
    # out.T -- the host transposes each sample's [K, E] result when gathering.
    out_d = nc.dram_tensor("out", [K, E], F32, kind="ExternalOutput")

    # feats DMA blocks (start chunk, n chunks): uniform 1MB blocks keep the
    # PE smoothly paced (2MB blocks make it idle in bursts, build a backlog,
    # then sprint at 100% duty -- earning a HAM half-clock penalty that
    # lands on the tail); the last block is split so the tail starts sooner.
    blocks_a = [(t, 4) for t in range(0, 28, 4)]
    blocks_b = [(28, 2), (30, 2)]

    with tile.TileContext(nc) as tc:
        with (
            tc.tile_pool(name="consts", bufs=1) as consts,
            tc.tile_pool(name="feats", bufs=9) as feats_pool,
            tc.tile_pool(name="small", bufs=4) as small,
            tc.tile_pool(name="outp", bufs=1) as outp,
            tc.tile_pool(name="ps_fs", bufs=1, space="PSUM") as ps_fs,
            tc.tile_pool(name="ps_sz", bufs=1, space="PSUM") as ps_sz,
            tc.tile_pool(name="ps_tr", bufs=1, space="PSUM") as ps_tr,
            tc.tile_pool(name="ps_misc", bufs=1, space="PSUM") as ps_misc,
            tc.tile_pool(name="ps_fin", bufs=1, space="PSUM") as ps_fin,
        ):
            # Bulk DMAs in FIFO order on the sync HWDGE queue.
            outputs_sb = consts.tile([P, OUT_AUG], F32)
            nc.sync.dma_start(out=outputs_sb, in_=outputs_d.ap())
            feats_r = feats_d.ap()
            wT_sb = consts.tile([P, FC, E], BF16)
            fgs = {}

            def load_feats(block_list):
                for t0, tb in block_list:
                    fg = feats_pool.tile([P, tb, FG, FGW], mm_dt,
                                         name=f"fg{t0}", tag="fg")
                    nc.sync.dma_start(out=fg, in_=feats_r[:, ds(t0, tb)])
                    fgs[t0] = fg

            load_feats(blocks_a)
            load_feats(blocks_b)
            nc.sync.dma_start(out=wT_sb, in_=wT_d.ap())
            blocks = blocks_a + blocks_b

            # All constants and the size/reciprocal computation are emitted
            # AFTER the stream so neither the in-order PE queue nor the
            # in-order DVE queue ever stalls the argmax or the first stream
            # chunks on work that is only needed by the tail.

            # Phase 1 (DVE only): onehot = (outT == rowmax) per hw chunk.
            oh_all = consts.tile([P, N_T, K], mm_dt)
            for t in range(N_T):
                rowmax = small.tile([P, 1], F32)
                nc.vector.tensor_reduce(
                    rowmax, outputs_sb[:, ds(t * K, K)], mybir.AxisListType.X,
                    mybir.AluOpType.max,
                )
                nc.vector.tensor_scalar(
                    out=oh_all[:, t, :],
                    in0=outputs_sb[:, ds(t * K, K)],
                    scalar1=rowmax,
                    scalar2=None,
                    op0=mybir.AluOpType.is_equal,
                )

            # Segment-reduce stream.  Per hw chunk: one size-count matmul
            # (onehot.T @ ones -> [21, 2]) plus four f-group matmuls packed
            # into the four PE column groups, accumulating [128, 512] PSUM
            # (f-group j at partitions 32j..32j+21) across all 32 chunks.
            fs_ps = ps_fs.tile([P, FGW], F32)
            for t0, tb in blocks:
                fg = fgs[t0]
                for ti in range(tb):
                    t = t0 + ti
                    oh_t = oh_all[:, t, :]
                    for j in range(FG):
                        nc.tensor.matmul(
                            fs_ps[ds(32 * j, K), :],
                            lhsT=oh_t,
                            rhs=fg[:, ti, j, :],
                            start=(t == 0),
                            stop=(t == N_T - 1),
                            tile_position=(0, 32 * j),
                        )

            # Tail constants (see note above the stream).
            ident = consts.tile([P, P], F32)
            make_identity(nc, ident)
            rep_sb = consts.tile([K, P], BF16)
            nc.vector.memset(rep_sb, 0.0)
            for j in range(FG):
                nc.vector.tensor_copy(rep_sb[:, ds(32 * j, K)], ident[:K, :K])
            ident21_b = consts.tile([K, K], BF16)
            nc.vector.tensor_copy(ident21_b, ident[:K, :K])
            identrep_ps = ps_tr.tile([P, K], F32, tag="t0")
            nc.tensor.matmul(
                identrep_ps, lhsT=rep_sb, rhs=ident21_b,
                start=True, stop=True,
            )
            ident_rep = consts.tile([P, K], BF16)
            nc.vector.tensor_copy(ident_rep, identrep_ps)
            one1_b = consts.tile([1, 1], BF16)
            nc.vector.memset(one1_b, 1.0)
            ones128_21 = consts.tile([P, K], BF16)
            nc.vector.memset(ones128_21, 1.0)
            ident_bf = consts.tile([P, P], BF16)
            nc.vector.tensor_copy(ident_bf, ident)
            # bias_diag[p, 128*ec + q] = bias[128*ec + p] * delta(p, q); a
            # matmul against all-ones gives final_ps[k, e] = bias[e].  It
            # runs mid-stream (own PSUM bank), so the tail's combine matmul
            # only accumulates on top.
            bias_diag = consts.tile([P, 2 * P], BF16)
            for ec in range(2):
                nc.vector.tensor_scalar_mul(
                    bias_diag[:, ds(ec * P, P)], ident_bf,
                    outputs_sb[:, N_T * K + ec : N_T * K + ec + 1],
                )
            final_ps = ps_fin.tile([K, E], F32, name="final")
            nc.tensor.matmul(final_ps, lhsT=ones128_21, rhs=bias_diag,
                             start=True, stop=False, skip_group_check=True)
            dummy_act = small.tile([1, 2], F32, tag="da")
            nc.scalar.activation(
                out=dummy_act, in_=outputs_sb[0:1, 0:2],
                func=mybir.ActivationFunctionType.Copy,
            )

            # Class sizes: reduce onehot over the hw-chunk axis on the DVE
            # (via a transposed free-dim view), then one fp32 matmul against
            # a ones column folds the partitions, giving the [1, 21] size
            # row.  The whole reciprocal chain completes during the stream,
            # so the tail starts scaling immediately.
            szk = small.tile([P, K], F32, tag="szk")
            nc.vector.tensor_reduce(
                szk, oh_all[:, :, :].rearrange("p t k -> p k t"),
                mybir.AxisListType.X, mybir.AluOpType.add,
            )
            ones_col = consts.tile([P, 1], F32)
            nc.vector.memset(ones_col, 1.0)
            szT_ps = ps_sz.tile([1, K], F32, tag="sz")
            nc.tensor.matmul(szT_ps, lhsT=ones_col, rhs=szk,
                             start=True, stop=True)
            sizesT = small.tile([1, K], F32, tag="sizesT")
            nc.vector.tensor_scalar_add(sizesT, szT_ps, 0.01)
            recipT = small.tile([1, K], F32, tag="recipT")
            nc.vector.reciprocal(recipT, sizesT)
            recipT_b = small.tile([1, K], BF16, tag="recipTb")
            nc.vector.tensor_copy(recipT_b, recipT)
            recipc_ps = ps_sz.tile([K, 1], F32, name="recipc", tag="sz")
            nc.tensor.matmul(recipc_ps, lhsT=recipT_b, rhs=one1_b,
                             start=True, stop=True)
            recip_b = small.tile([K, 1], BF16, tag="recipb")
            nc.vector.tensor_copy(recip_b, recipc_ps)
            recip_ps = ps_sz.tile([P, 1], F32, name="recip128", tag="sz")
            nc.tensor.matmul(recip_ps, lhsT=rep_sb, rhs=recip_b,
                             start=True, stop=True)
            recip128 = small.tile([P, 1], F32, tag="r128")
            nc.vector.tensor_copy(recip128, recip_ps)

            # Scale by 1/sizes during the PSUM->SBUF copy (DVE + ACT halves).
            fs_sc = consts.tile([P, FGW], BF16)
            nc.vector.tensor_scalar_mul(
                fs_sc[:, 0 : FGW // 2], fs_ps[:, 0 : FGW // 2], recip128
            )
            nc.scalar.activation(
                out=fs_sc[:, ds(FGW // 2, FGW // 2)],
                in_=fs_ps[:, ds(FGW // 2, FGW // 2)],
                func=mybir.ActivationFunctionType.Copy,
                scale=recip128,
            )

            # Transposes, 4 concurrent per round (one per 32-row row group),
            # all into one PSUM tile, drained by two half-copies.  f-chunk
            # fc = 4j + c lives in column group j at free cols 128c..128c+128
            # of fs_sc.
            # (stride padded to 22 elements so each bf16 PSUM write is
            # 4-byte aligned)
            # NOTE: transpose-mode matmuls writing at a nonzero PSUM free
            # offset hang the hardware (verified twice) -- each transpose
            # gets its own PSUM slot, drained per-chunk on alternating
            # engines.
            fsT_sb = consts.tile([P, FC, K], BF16)
            for c in range(4):
                trps = []
                for j in range(FG):
                    fc = 4 * j + c
                    trp = ps_tr.tile([P, K], BF16, name=f"trp{fc}",
                                     tag=f"t{j}")
                    nc.tensor.transpose(
                        trp,
                        fs_sc[ds(32 * j, K), ts(c, P)],
                        ident_rep[ds(32 * j, K), :],
                        tile_position=(32 * j, 0),
                    )
                    trps.append((fc, trp))
                for i, (fc, trp) in enumerate(trps):
                    if i % 2 == 0:
                        nc.vector.tensor_copy(fsT_sb[:, fc, :], trp)
                    else:
                        nc.scalar.activation(
                            out=fsT_sb[:, fc, :], in_=trp,
                            func=mybir.ActivationFunctionType.Copy,
                        )

            # Projection, 4 concurrent per round: round r takes one f-chunk
            # from each column group (fc = 4j + r), so it depends only on
            # transpose round r -- transposes, drain copies and projection
            # rounds pipeline instead of the projections waiting for the
            # whole transpose pass.
            proj_ps = ps_misc.tile([P, E], F32, tag="warm", name="proj")
            for r in range(4):
                for j in range(FG):
                    fc = 4 * j + r
                    nc.tensor.matmul(
                        proj_ps[ds(32 * j, K), :],
                        lhsT=fsT_sb[:, fc, :],
                        rhs=wT_sb[:, fc, :],
                        start=(r == 0),
                        stop=(r == 3),
                        tile_position=(0, 32 * j),
                    )
            proj_sb = consts.tile([P, E], BF16)
            nc.vector.tensor_copy(proj_sb[:, 0 : E // 2], proj_ps[:, 0 : E // 2])
            nc.scalar.activation(
                out=proj_sb[:, ds(E // 2, E // 2)],
                in_=proj_ps[:, ds(E // 2, E // 2)],
                func=mybir.ActivationFunctionType.Copy,
            )

            # Combine the four partials onto the bias already in final_ps:
            # final[k, e] = bias[e] + sum_j proj[32j+k, e].
            nc.tensor.matmul(final_ps, lhsT=ident_rep, rhs=proj_sb,
                             start=False, stop=True, skip_group_check=True)
            out_sb = outp.tile([K, E], F32)
            nc.vector.tensor_copy(out_sb[:, 0 : E // 2], final_ps[:, 0 : E // 2])
            nc.scalar.activation(
                out=out_sb[:, ds(E // 2, E // 2)],
                in_=final_ps[:, ds(E // 2, E // 2)],
                func=mybir.ActivationFunctionType.Copy,
            )
            nc.sync.dma_start(out=out_d.ap(), in_=out_sb)

    nc.compile()
    return nc


_CACHE = {}


def make_in_maps(outputs, feats, w_proj, b_proj, dtype=DTYPE):
    import ml_dtypes

    mm_np = ml_dtypes.float8_e3m4 if dtype == "fp8" else ml_dtypes.bfloat16
    outputs = np.asarray(outputs, dtype=np.float32)
    outputs_t = outputs.reshape(B, K, N_T, P).transpose(0, 3, 2, 1).reshape(
        B, P, N_T * K
    )
    bias = np.asarray(b_proj, dtype=np.float32).reshape(2, P).T  # [p, ec]
    outputs_aug = np.ascontiguousarray(
        np.concatenate([outputs_t, np.broadcast_to(bias, (B, P, 2))], axis=2)
    )
    feats = np.asarray(feats, dtype=np.float32).astype(mm_np)
    # [B, F, H, W] -> per sample [p, t, fgrp, fj] = featsT[t*128+p, fgrp*512+fj]
    feats_sh = np.ascontiguousarray(
        feats.reshape(B, FG, FGW, N_T, P).transpose(0, 4, 3, 1, 2)
    )
    # w_proj [E, F] -> wT [p, fc, e] = w_proj.T[fc*128+p, e]
    wT = np.ascontiguousarray(
        np.asarray(w_proj, dtype=np.float32)
        .T.astype(ml_dtypes.bfloat16)
        .reshape(FC, P, E)
        .transpose(1, 0, 2)
    )
    return [
        {
            "outputs_in": outputs_aug[b],
            "feats_in": feats_sh[b],
            "wT_in": wT,
        }
        for b in range(B)
    ]


def kernel(outputs, feats, w_proj, b_proj, _trace=False, _trace_kwargs=None,
           _dtype=DTYPE, _build_kwargs=None):
    key = (_dtype, tuple(sorted((_build_kwargs or {}).items())))
    if key not in _CACHE:
        _CACHE[key] = build_module(dtype=_dtype, **(_build_kwargs or {}))
    nc = _CACHE[key]
    in_maps = make_in_maps(outputs, feats, w_proj, b_proj, dtype=_dtype)
    res = run_bass_kernel_spmd(
        nc,
        in_maps,
        core_ids=list(range(N_CORES)),
        trace=_trace,
        **(_trace_kwargs or {}),
    )
    # each core returns out.T [K, E]; transpose back to [E, K] and stack
    out = np.stack([np.asarray(r["out"]).T for r in res.results])
    if _trace:
        _CACHE["last_results"] = res
    return out


# revision 35
# speedup vs baseline: 1.0843x; 1.0843x over previous
"""Trainium2 Bass kernel for nn_Encoder segment-reduce.

Reference computation (per sample b):
    cls = onehot(argmax_k outputs[b])            # [K, HW]
    sizes = cls.sum(HW) + 0.01                   # [K]
    feat_set = feats[b] @ cls.T / sizes          # [F, K]
    out[b] = w_proj @ feat_set + bias            # [E, K]

Kernel strategy (pure data parallel: 1 sample per NeuronCore, 8 cores).

Segment-reduce FIRST (the cheap contraction), projection second:
    feat_setT[k, f] = sum_hw onehot[hw, k] * featsT[hw, f]
with the onehot chunk [128hw, 21] as the PE's stationary operand and featsT
chunks [128hw, 512f] as the moving operand.  The four f-group matmuls of each
hw chunk are packed into the four 32-column groups of the PE array via
tile_position=(0, 32j): the stationary onehot only occupies 21 of 128 array
columns, so the four matmuls execute concurrently (measured 4ns stagger) and
the stream keeps pace with the DMA.  One [128, 512] PSUM tile holds all four
accumulators (f-group j at partitions 32j..32j+21).

feats dtype is fp8 e3m4 (TRN FP8_EXP3): N(0,1) data fits the e3m4 range and
its 4 mantissa bits give rel err ~1.2e-2 end to end (threshold 2e-2), halving
HBM traffic vs bf16: 8.39 MB feats + 1 MB wT (bf16) + 0.34 MB outputs (f32)
~= 9.75 MB/core -- the kernel is DMA-bound at the per-core HBM limit.  The
matmuls run in normal (single-rate) fp8 mode: double-fp8 would upcast
operands to e6m3 and destroy e3m4's 4th mantissa bit (measured 3.0e-2).

All host-side layouts give every DMA >= 2KB contiguous per-partition runs
(wT is pre-permuted to [p, fc, e] -- the naive [F, E] order DMAs in 512B
pieces at ~60% efficiency; the bias rides inside the outputs transfer).
DMA order: outputs+bias first (the DVE argmax fills the initial feats
window), feats in uniform 1MB blocks (2MB blocks make the PE idle in
bursts, build a backlog, then sprint at 100% duty -- earning a HAM
half-clock throttle that lands on the tail), then wT (needed only by the
projection).  All constants, the class sizes (a DVE reduce over a
transposed view of the onehot + one partition-folding matmul) and the
whole reciprocal chain are emitted AFTER the stream in program order:
both the PE and the DVE execute in order, so anything emitted earlier
stalls the argmax / the first stream chunks (the scheduler hoists the
emitted-late constants into idle windows instead).  No dummy warm-up
matmuls: the HAM clock governor charges full-array dummy work back as a
half-clock debt on the real work, and the DMA-paced stream keeps pace
even at the cold 1.2 GHz clock.

Tail (all PE stages packed into array tile groups):
  - 1/sizes (ready mid-stream) is fused into the PSUM->SBUF copies of the
    accumulators (DVE + ACT halves);
  - the 16 [21,128]->[128,21] transposes run 4-at-a-time (one per 32-row
    row group, via a replicated identity) into the four bank-aligned
    sections of one PSUM tile, each round drained by a single strided
    copy (NOTE: transpose-mode matmuls writing at a nonzero offset WITHIN
    a PSUM bank hang the hardware -- verified twice);
  - the 16 projection matmuls (feat_set chunk stationary, wT moving) run
    4-at-a-time in the four column groups; round r takes f-chunk 4j+r of
    group j so it depends only on transpose round r, and the four partial
    [21, 256] sums land at partitions 32j; one final matmul against the
    replicated identity folds them onto the bias, which a diagonal-bias
    matmul parked in the accumulator mid-stream.
The output is stored as [K, E]; the host transposes when gathering.

dtype: "fp8" (e3m4 feats, rel err ~1.2e-2) or "bf16" (rel err ~3e-3).
"""

import numpy as np

import concourse.bacc as bacc
import concourse.bass as bass
import concourse.mybir as mybir
import concourse.tile as tile
from concourse.bass import ds, ts
from concourse.bass_utils import run_bass_kernel_spmd
from concourse.masks import make_identity

# Problem shapes (hardcoded per contract)
B = 8
K = 21
H = 64
W = 64
HW = H * W            # 4096
F = 2048
E = 256
P = 128
FC = F // P           # 16 f-chunks of 128
FG = 4                # f-groups of 512 (PE column groups)
FGW = F // FG         # 512
N_T = HW // P         # 32 hw chunks
OUT_AUG = N_T * K + 2  # outputs row + 2 bias values per partition
N_CORES = 8

F32 = mybir.dt.float32
BF16 = mybir.dt.bfloat16
FP8 = mybir.dt.float8e3   # e3m4: 4 mantissa bits

DTYPE = "fp8"         # "fp8" or "bf16"


def build_module(dtype=DTYPE):
    mm_dt = FP8 if dtype == "fp8" else BF16
    nc = bacc.Bacc("TRN2", target_bir_lowering=False, debug=False)

    # outputs host-transposed to [p, t*k] (pixel-major), with the two bias
    # values of partition p (bias[p], bias[128+p]) appended -- one transfer.
    outputs_d = nc.dram_tensor("outputs_in", [P, OUT_AUG], F32, kind="ExternalInput")
    # featsT host-permuted to [p, t, fgrp, fj]: featsT[t*128+p, fgrp*512+fj].
    feats_d = nc.dram_tensor(
        "feats_in", [P, N_T, FG, FGW], mm_dt, kind="ExternalInput"
    )
    # wT host-permuted to [p, fc, e] = w_proj.T[fc*128+p, e] (contiguous 8KB
    # per-partition runs -- the naive [F, E] layout DMAs in 512B pieces).
    wT_d = nc.dram_tensor("wT_in", [P, FC, E], BF16, kind="ExternalInput")
    # out.T -- the host transposes each sample's [K, E] result when gathering.
    out_d = nc.dram_tensor("out", [K, E], F32, kind="ExternalOutput")

    # feats DMA blocks (start chunk, n chunks): uniform 1MB blocks keep the
    # PE smoothly paced (2MB blocks make it idle in bursts, build a backlog,
    # then sprint at 100% duty -- earning a HAM half-clock penalty that
    # lands on the tail); the last block is split so the tail starts sooner.
    blocks_a = [(t, 4) for t in range(0, 28, 4)]
    blocks_b = [(28, 2), (30, 2)]

    with tile.TileContext(nc) as tc:
        with (
            tc.tile_pool(name="consts", bufs=1) as consts,
            tc.tile_pool(name="feats", bufs=9) as feats_pool,
            tc.tile_pool(name="small", bufs=4) as small,
            tc.tile_pool(name="outp", bufs=1) as outp,
            tc.tile_pool(name="ps_fs", bufs=1, space="PSUM") as ps_fs,
            tc.tile_pool(name="ps_sz", bufs=1, space="PSUM") as ps_sz,
            tc.tile_pool(name="ps_tr", bufs=1, space="PSUM") as ps_tr,
            tc.tile_pool(name="ps_misc", bufs=1, space="PSUM") as ps_misc,
            tc.tile_pool(name="ps_fin", bufs=1, space="PSUM") as ps_fin,
        ):
            # Bulk DMAs in FIFO order on the sync HWDGE queue.
            outputs_sb = consts.tile([P, OUT_AUG], F32)
            nc.sync.dma_start(out=outputs_sb, in_=outputs_d.ap())
            feats_r = feats_d.ap()
            wT_sb = consts.tile([P, FC, E], BF16)
            fgs = {}

            def load_feats(block_list):
                for t0, tb in block_list:
                    fg = feats_pool.tile([P, tb, FG, FGW], mm_dt,
                                         name=f"fg{t0}", tag="fg")
                    nc.sync.dma_start(out=fg, in_=feats_r[:, ds(t0, tb)])
                    fgs[t0] = fg

            load_feats(blocks_a)
            load_feats(blocks_b)
            nc.sync.dma_start(out=wT_sb, in_=wT_d.ap())
            blocks = blocks_a + blocks_b

            # All constants and the size/reciprocal computation are emitted
            # AFTER the stream so neither the in-order PE queue nor the
            # in-order DVE queue ever stalls the argmax or the first stream
            # chunks on work that is only needed by the tail.

            # Phase 1 (DVE only): onehot = (outT == rowmax) per hw chunk.
            oh_all = consts.tile([P, N_T, K], mm_dt)
            for t in range(N_T):
                rowmax = small.tile([P, 1], F32)
                nc.vector.tensor_reduce(
                    rowmax, outputs_sb[:, ds(t * K, K)], mybir.AxisListType.X,
                    mybir.AluOpType.max,
                )
                nc.vector.tensor_scalar(
                    out=oh_all[:, t, :],
                    in0=outputs_sb[:, ds(t * K, K)],
                    scalar1=rowmax,
                    scalar2=None,
                    op0=mybir.AluOpType.is_equal,
                )

            # Segment-reduce stream.  Per hw chunk: one size-count matmul
            # (onehot.T @ ones -> [21, 2]) plus four f-group matmuls packed
            # into the four PE column groups, accumulating [128, 512] PSUM
            # (f-group j at partitions 32j..32j+21) across all 32 chunks.
            fs_ps = ps_fs.tile([P, FGW], F32)
            for t0, tb in blocks:
                fg = fgs[t0]
                for ti in range(tb):
                    t = t0 + ti
                    oh_t = oh_all[:, t, :]
                    for j in range(FG):
                        nc.tensor.matmul(
                            fs_ps[ds(32 * j, K), :],
                            lhsT=oh_t,
                            rhs=fg[:, ti, j, :],
                            start=(t == 0),
                            stop=(t == N_T - 1),
                            tile_position=(0, 32 * j),
                        )

            # Tail constants (see note above the stream).
            ident = consts.tile([P, P], F32)
            make_identity(nc, ident)
            rep_sb = consts.tile([K, P], BF16)
            nc.vector.memset(rep_sb, 0.0)
            for j in range(FG):
                nc.vector.tensor_copy(rep_sb[:, ds(32 * j, K)], ident[:K, :K])
            ident21_b = consts.tile([K, K], BF16)
            nc.vector.tensor_copy(ident21_b, ident[:K, :K])
            identrep_ps = ps_tr.tile([P, K], F32, tag="t0")
            nc.tensor.matmul(
                identrep_ps, lhsT=rep_sb, rhs=ident21_b,
                start=True, stop=True,
            )
            ident_rep = consts.tile([P, K], BF16)
            nc.vector.tensor_copy(ident_rep, identrep_ps)
            one1_b = consts.tile([1, 1], BF16)
            nc.vector.memset(one1_b, 1.0)
            ones128_21 = consts.tile([P, K], BF16)
            nc.vector.memset(ones128_21, 1.0)
            ident_bf = consts.tile([P, P], BF16)
            nc.vector.tensor_copy(ident_bf, ident)
            # bias_diag[p, 128*ec + q] = bias[128*ec + p] * delta(p, q); a
            # matmul against all-ones gives final_ps[k, e] = bias[e].  It
            # runs mid-stream (own PSUM bank), so the tail's combine matmul
            # only accumulates on top.
            bias_diag = consts.tile([P, 2 * P], BF16)
            for ec in range(2):
                nc.vector.tensor_scalar_mul(
                    bias_diag[:, ds(ec * P, P)], ident_bf,
                    outputs_sb[:, N_T * K + ec : N_T * K + ec + 1],
                )
            final_ps = ps_fin.tile([K, E], F32, name="final")
            nc.tensor.matmul(final_ps, lhsT=ones128_21, rhs=bias_diag,
                             start=True, stop=False, skip_group_check=True)
            dummy_act = small.tile([1, 2], F32, tag="da")
            nc.scalar.activation(
                out=dummy_act, in_=outputs_sb[0:1, 0:2],
                func=mybir.ActivationFunctionType.Copy,
            )

            # Class sizes: reduce onehot over the hw-chunk axis on the DVE
            # (via a transposed free-dim view), then one fp32 matmul against
            # a ones column folds the partitions, giving the [1, 21] size
            # row.  The whole reciprocal chain completes during the stream,
            # so the tail starts scaling immediately.
            szk = small.tile([P, K], F32, tag="szk")
            nc.vector.tensor_reduce(
                szk, oh_all[:, :, :].rearrange("p t k -> p k t"),
                mybir.AxisListType.X, mybir.AluOpType.add,
            )
            ones_col = consts.tile([P, 1], F32)
            nc.vector.memset(ones_col, 1.0)
            szT_ps = ps_sz.tile([1, K], F32, tag="sz")
            nc.tensor.matmul(szT_ps, lhsT=ones_col, rhs=szk,
                             start=True, stop=True)
            sizesT = small.tile([1, K], F32, tag="sizesT")
            nc.vector.tensor_scalar_add(sizesT, szT_ps, 0.01)
            recipT = small.tile([1, K], F32, tag="recipT")
            nc.vector.reciprocal(recipT, sizesT)
            recipT_b = small.tile([1, K], BF16, tag="recipTb")
            nc.vector.tensor_copy(recipT_b, recipT)
            recipc_ps = ps_sz.tile([K, 1], F32, name="recipc", tag="sz")
            nc.tensor.matmul(recipc_ps, lhsT=recipT_b, rhs=one1_b,
                             start=True, stop=True)
            recip_b = small.tile([K, 1], BF16, tag="recipb")
            nc.vector.tensor_copy(recip_b, recipc_ps)
            recip_ps = ps_sz.tile([P, 1], F32, name="recip128", tag="sz")
            nc.tensor.matmul(recip_ps, lhsT=rep_sb, rhs=recip_b,
                             start=True, stop=True)
            recip128 = small.tile([P, 1], F32, tag="r128")
            nc.vector.tensor_copy(recip128, recip_ps)

            # Scale by 1/sizes during the PSUM->SBUF copy (DVE + ACT halves).
            fs_sc = consts.tile([P, FGW], BF16)
            nc.vector.tensor_scalar_mul(
                fs_sc[:, 0 : FGW // 2], fs_ps[:, 0 : FGW // 2], recip128
            )
            nc.scalar.activation(
                out=fs_sc[:, ds(FGW // 2, FGW // 2)],
                in_=fs_ps[:, ds(FGW // 2, FGW // 2)],
                func=mybir.ActivationFunctionType.Copy,
                scale=recip128,
            )

            # Transposes, 4 concurrent per round (one per 32-row row group),
            # all into one PSUM tile, drained by two half-copies.  f-chunk
            # fc = 4j + c lives in column group j at free cols 128c..128c+128
            # of fs_sc.
            # (stride padded to 22 elements so each bf16 PSUM write is
            # 4-byte aligned)
            # NOTE: transpose-mode matmuls writing at a nonzero offset
            # WITHIN a PSUM bank hang the hardware (verified twice); writes
            # at bank-aligned offsets are fine.  One 4-bank PSUM tile holds
            # a whole transpose round (column group j's output at bank j),
            # so each round drains with a single strided copy, alternating
            # engines -- 4 copies total instead of 16.
            trp_rnd = ps_tr.tile([P, FG, 1024], BF16, tag="t0", name="trp")
            fsT_sb = consts.tile([P, FC, K], BF16)
            for c in range(4):
                for j in range(FG):
                    nc.tensor.transpose(
                        trp_rnd[:, j, 0:K],
                        fs_sc[ds(32 * j, K), ts(c, P)],
                        ident_rep[ds(32 * j, K), :],
                        tile_position=(32 * j, 0),
                    )
                # fc = 4j + c: round c's four chunks interleave into fsT
                # with free stride 4*K.
                dst = fsT_sb[:, :, :].rearrange("p (j c) k -> p c j k", c=4)
                if c % 2 == 0:
                    nc.vector.tensor_copy(dst[:, c], trp_rnd[:, :, 0:K])
                else:
                    nc.scalar.activation(
                        out=dst[:, c], in_=trp_rnd[:, :, 0:K],
                        func=mybir.ActivationFunctionType.Copy,
                    )

            # Projection, 4 concurrent per round: round r takes one f-chunk
            # from each column group (fc = 4j + r), so it depends only on
            # transpose round r -- transposes, drain copies and projection
            # rounds pipeline instead of the projections waiting for the
            # whole transpose pass.
            proj_ps = ps_misc.tile([P, E], F32, tag="warm", name="proj")
            for r in range(4):
                for j in range(FG):
                    fc = 4 * j + r
                    nc.tensor.matmul(
                        proj_ps[ds(32 * j, K), :],
                        lhsT=fsT_sb[:, fc, :],
                        rhs=wT_sb[:, fc, :],
                        start=(r == 0),
                        stop=(r == 3),
                        tile_position=(0, 32 * j),
                    )
            proj_sb = consts.tile([P, E], BF16)
            nc.vector.tensor_copy(proj_sb[:, 0 : E // 2], proj_ps[:, 0 : E // 2])
            nc.scalar.activation(
                out=proj_sb[:, ds(E // 2, E // 2)],
                in_=proj_ps[:, ds(E // 2, E // 2)],
                func=mybir.ActivationFunctionType.Copy,
            )

            # Combine the four partials onto the bias already in final_ps:
            # final[k, e] = bias[e] + sum_j proj[32j+k, e].
            nc.tensor.matmul(final_ps, lhsT=ident_rep, rhs=proj_sb,
                             start=False, stop=True, skip_group_check=True)
            out_sb = outp.tile([K, E], F32)
            nc.vector.tensor_copy(out_sb[:, 0 : E // 2], final_ps[:, 0 : E // 2])
            nc.scalar.activation(
                out=out_sb[:, ds(E // 2, E // 2)],
                in_=final_ps[:, ds(E // 2, E // 2)],
                func=mybir.ActivationFunctionType.Copy,
            )
            nc.sync.dma_start(out=out_d.ap(), in_=out_sb)

    nc.compile()
    return nc


_CACHE = {}


def make_in_maps(outputs, feats, w_proj, b_proj, dtype=DTYPE):
    import ml_dtypes

    mm_np = ml_dtypes.float8_e3m4 if dtype == "fp8" else ml_dtypes.bfloat16
    outputs = np.asarray(outputs, dtype=np.float32)
    outputs_t = outputs.reshape(B, K, N_T, P).transpose(0, 3, 2, 1).reshape(
        B, P, N_T * K
    )
    bias = np.asarray(b_proj, dtype=np.float32).reshape(2, P).T  # [p, ec]
    outputs_aug = np.ascontiguousarray(
        np.concatenate([outputs_t, np.broadcast_to(bias, (B, P, 2))], axis=2)
    )
    feats = np.asarray(feats, dtype=np.float32).astype(mm_np)
    # [B, F, H, W] -> per sample [p, t, fgrp, fj] = featsT[t*128+p, fgrp*512+fj]
    feats_sh = np.ascontiguousarray(
        feats.reshape(B, FG, FGW, N_T, P).transpose(0, 4, 3, 1, 2)
    )
    # w_proj [E, F] -> wT [p, fc, e] = w_proj.T[fc*128+p, e]
    wT = np.ascontiguousarray(
        np.asarray(w_proj, dtype=np.float32)
        .T.astype(ml_dtypes.bfloat16)
        .reshape(FC, P, E)
        .transpose(1, 0, 2)
    )
    return [
        {
            "outputs_in": outputs_aug[b],
            "feats_in": feats_sh[b],
            "wT_in": wT,
        }
        for b in range(B)
    ]


def kernel(outputs, feats, w_proj, b_proj, _trace=False, _trace_kwargs=None,
           _dtype=DTYPE, _build_kwargs=None):
    key = (_dtype, tuple(sorted((_build_kwargs or {}).items())))
    if key not in _CACHE:
        _CACHE[key] = build_module(dtype=_dtype, **(_build_kwargs or {}))
    nc = _CACHE[key]
    in_maps = make_in_maps(outputs, feats, w_proj, b_proj, dtype=_dtype)
    res = run_bass_kernel_spmd(
        nc,
        in_maps,
        core_ids=list(range(N_CORES)),
        trace=_trace,
        **(_trace_kwargs or {}),
    )
    # each core returns out.T [K, E]; transpose back to [E, K] and stack
    out = np.stack([np.asarray(r["out"]).T for r in res.results])
    if _trace:
        _CACHE["last_results"] = res
    return out
